# revision 10
# baseline (speedup 1.0000x reference)
"""Hybrid Strassen TRN2 kernel for MockFP8Linear: out = x @ (W*scale)^T.

Per core: C [2048,2048] = A @ B, A = x-shard, B = (W*scale)^T.
Token rows tt0/tt8 are computed as a PLAIN chunk-pair matmul stream first
(no combos needed -- it runs while the B-side Strassen operands are being
dequantized and combined), then one Strassen level handles the remaining
7 row-pair passes: 4*32 + 7*112 = 912 N=512 matmuls vs 1024 plain (-10.9%).

  M1=(A11+A22)(B11+B22)  M2=(A21+A22)B11      M3=A11(B12-B22)
  M4=A22(B21-B11)        M5=(A11+A12)B22      M6=(A21-A11)(B11+B12)
  M7=(A12-A22)(B21+B22)
  C11=M1+M4-M5+M7  C12=M3+M5  C21=M2+M4  C22=M1-M2+M3+M6

All matmuls are PAIRED: one stationary lhsT feeds 2 matmuls into 2
alternating PSUM banks (per-matmul stationary swaps measured +43 ns each).
Process order per Strassen pass: [M2, M5, M1, M7, M4, M3, M6]; each
C-quadrant accumulator's first contribution is a copy (ACT) or negated
copy (DVE tensor_scalar_mul), later ones accumulate in place on DVE.

B quadrants (r = i-half, q = o-half): B_rq = wsb[ch in {2q,2q+1}][ib in
8r..8r+8). cB3=B12-B22 / cB4=B21-B11 are computed IN PLACE over the raw
B12/B21 slabs (raw values are only read by the plain pass and cb6/cb7,
which Tile orders first). cb1/cb6/cb7 get their own tiles (6 MB).
Engine budget: DVE = dequants, cb4/cb6/cb3, A-combos, accumulate ops;
GPSIMD = cb1/cb7 (+output DMA issues); ACT = plain copies (+output DMAs).
x slabs stream through a 4-buffer pool (each pass uses only tt', tt'+8).
"""

import os
import sys

import numpy as np

for _p in ("/opt/trn_rl_repo", "/root/.axon_site/_ro/trn_rl_repo"):
    if os.path.isdir(_p) and _p not in sys.path:
        sys.path.append(_p)

TOKENS, IN_F, OUT_F = 16384, 2048, 2048
NCORES = 8
TSH = TOKENS // NCORES
P = 128
KB = IN_F // P
TB = TSH // P
OBL = OUT_F // P
NCH = OUT_F // 512

_cached = None


def _build():
    from contextlib import ExitStack

    import concourse.tile as tile
    from concourse import bacc, mybir
    from concourse.bass import ds

    f32 = mybir.dt.float32
    bf16 = mybir.dt.bfloat16
    add = mybir.AluOpType.add
    sub = mybir.AluOpType.subtract
    mult = mybir.AluOpType.mult

    nc = bacc.Bacc("TRN2", target_bir_lowering=False, debug=False, num_devices=NCORES)
    xt_d = nc.dram_tensor("xt", [TB, P, KB, P], bf16, kind="ExternalInput").ap()
    wt_d = nc.dram_tensor("wt", [NCH, KB, P, 512], bf16, kind="ExternalInput").ap()
    s_d = nc.dram_tensor("s", [P, KB, OBL], bf16, kind="ExternalInput").ap()
    o_d = nc.dram_tensor("out", [TSH, OUT_F], f32, kind="ExternalOutput").ap()

    with tile.TileContext(nc) as tc:
        with ExitStack() as ctx:
            const = ctx.enter_context(tc.tile_pool(name="const", bufs=1))
            scales = const.tile([P, KB, OBL], bf16)
            nc.sync.dma_start(scales[:], s_d[:])

            w_pool = ctx.enter_context(tc.tile_pool(name="w", bufs=1))
            wsb = [
                [w_pool.tile([P, 512], bf16, name=f"w_{ch}_{ib}") for ib in range(KB)]
                for ch in range(NCH)
            ]
            cb_pool = ctx.enter_context(tc.tile_pool(name="cb", bufs=1))
            cb1 = [
                [cb_pool.tile([P, 512], bf16, name=f"cb1_{j}_{q}") for q in range(2)]
                for j in range(8)
            ]
            cb6 = [
                [cb_pool.tile([P, 512], bf16, name=f"cb6_{j}_{q}") for q in range(2)]
                for j in range(8)
            ]
            cb7 = [
                [cb_pool.tile([P, 512], bf16, name=f"cb7_{j}_{q}") for q in range(2)]
                for j in range(8)
            ]

            x_pool = ctx.enter_context(tc.tile_pool(name="x", bufs=4))
            am_pool = ctx.enter_context(tc.tile_pool(name="am", bufs=10))
            osb_pool = ctx.enter_context(tc.tile_pool(name="osb", bufs=12))
            osbp_pool = ctx.enter_context(tc.tile_pool(name="osbp", bufs=4))
            ps_pool = ctx.enter_context(tc.tile_pool(name="ps", bufs=8, space="PSUM"))

            # PE warmup (flips the HAM clock gate during the preamble)
            wm_ps = ps_pool.tile([16, 256], f32, tag="ps", name="warm")
            for _ in range(7):
                nc.tensor.matmul(
                    wm_ps[:], lhsT=scales[:, 0, :], rhs=scales[:, :, :],
                    start=True, stop=True,
                )

            def load_w(ch, ib):
                nc.sync.dma_start(wsb[ch][ib][:], wt_d[ch, ib])
                nc.vector.tensor_tensor(
                    out=wsb[ch][ib][:].rearrange("p (b c) -> p b c", c=P),
                    in0=wsb[ch][ib][:].rearrange("p (b c) -> p b c", c=P),
                    in1=scales[:, ib, ds(ch * 4, 4), None].broadcast_to([P, 4, P]),
                    op=mult,
                )

            B11 = lambda j, q: wsb[q][j]
            B21 = lambda j, q: wsb[q][8 + j]
            B22 = lambda j, q: wsb[2 + q][8 + j]
            B12 = lambda j, q: wsb[2 + q][j]

            # ---- W DMA+dequant in chunk-pair JIT order (plain pass shape),
            # x slabs 0/8 woven in early ----
            load_w(0, 0)
            load_w(1, 0)
            xcur = {}
            for t in (0, 8):
                xcur[t] = x_pool.tile([P, KB, P], bf16, tag="x", name=f"x_{t}")
            nc.sync.dma_start(xcur[0][:], xt_d[0])
            for ib in range(1, KB):
                load_w(0, ib)
                load_w(1, ib)
                if ib == 5:
                    nc.sync.dma_start(xcur[8][:], xt_d[8])
            for ch in (2, 3):
                for ib in range(KB):
                    load_w(ch, ib)

            # ---- plain chunk-pair pass over token tiles 0 and 8 ----
            def plain_group(chp, tt, xa):
                psum = [
                    ps_pool.tile([P, 512], f32, tag="ps", name=f"pp_{chp}_{tt}_{k}")
                    for k in range(2)
                ]
                for ib in range(KB):
                    for k in range(2):
                        nc.tensor.matmul(
                            psum[k][:],
                            lhsT=xa[:, ib, :],
                            rhs=wsb[2 * chp + k][ib][:],
                            start=(ib == 0),
                            stop=(ib == KB - 1),
                        )
                osb = osbp_pool.tile([P, 1024], f32, tag="osbp", name=f"op_{chp}_{tt}")
                nc.scalar.copy(osb[:, ds(0, 512)], psum[0][:])
                nc.scalar.copy(osb[:, ds(512, 512)], psum[1][:])
                nc.gpsimd.dma_start(o_d[ds(tt * P, P), ds(chp * 1024, 1024)], osb[:])

            for chp in range(2):
                for tt in (0, 8):
                    plain_group(chp, tt, xcur[tt])

            # prefetch x for strassen pass 1
            for t in (1, 9):
                xcur[t] = x_pool.tile([P, KB, P], bf16, tag="x", name=f"x_{t}")
                nc.sync.dma_start(xcur[t][:], xt_d[t])

            # ---- B-combos (built while the plain pass runs) ----
            # GPSIMD: cb1, cb7 (pure reads). DVE: cb4, cb6, cb3 (cb4/cb3 in
            # place; Tile orders them after the plain pass + cb6/cb7 reads).
            for j in range(8):
                for q in range(2):
                    nc.gpsimd.tensor_tensor(
                        out=cb1[j][q][:], in0=B11(j, q)[:], in1=B22(j, q)[:], op=add
                    )
            for j in range(8):
                for q in range(2):
                    nc.gpsimd.tensor_tensor(
                        out=cb7[j][q][:], in0=B21(j, q)[:], in1=B22(j, q)[:], op=add
                    )
            for j in range(8):
                for q in range(2):
                    nc.vector.tensor_tensor(
                        out=B21(j, q)[:], in0=B21(j, q)[:], in1=B11(j, q)[:], op=sub
                    )  # cb4 (after cb7 read raw B21)
            for j in range(8):
                for q in range(2):
                    nc.vector.tensor_tensor(
                        out=cb6[j][q][:], in0=B11(j, q)[:], in1=B12(j, q)[:], op=add
                    )
            for j in range(8):
                for q in range(2):
                    nc.vector.tensor_tensor(
                        out=B12(j, q)[:], in0=B12(j, q)[:], in1=B22(j, q)[:], op=sub
                    )  # cb3 (after cb6 read raw B12)

            def rhs(m, j, q):
                return {
                    1: cb1[j][q], 2: B11(j, q), 3: B12(j, q), 4: B21(j, q),
                    5: B22(j, q), 6: cb6[j][q], 7: cb7[j][q],
                }[m]

            def build_am(tp, xa, xb):
                a11 = xa[:, ds(0, 8), :]
                a12 = xa[:, ds(8, 8), :]
                a21 = xb[:, ds(0, 8), :]
                a22 = xb[:, ds(8, 8), :]
                am = {3: a11, 4: a22}
                for m, (i0, i1, op) in {
                    1: (a11, a22, add),
                    2: (a21, a22, add),
                    5: (a11, a12, add),
                    6: (a21, a11, sub),
                    7: (a12, a22, sub),
                }.items():
                    t = am_pool.tile([P, 8, P], bf16, tag="am", name=f"am_{tp}_{m}")
                    nc.vector.tensor_tensor(out=t[:], in0=i0, in1=i1, op=op)
                    am[m] = t
                return am

            am_by_tp = {1: build_am(1, xcur[1], xcur[9])}

            # C assembly: osb 0=C11, 1=C12, 2=C21, 3=C22
            recipe = {
                1: [(0, +1), (3, +1)],
                2: [(2, +1), (3, -1)],
                3: [(1, +1), (3, +1)],
                4: [(0, +1), (2, +1)],
                5: [(0, -1), (1, +1)],
                6: [(3, +1)],
                7: [(0, +1)],
            }
            m_order = [2, 5, 1, 7, 4, 3, 6]
            n_ops = {0: 4, 1: 2, 2: 2, 3: 4}

            def out_ap(qd, tp, q):
                r0 = tp * P + (1024 if qd >= 2 else 0)
                c0 = q * 512 + (1024 if qd % 2 == 1 else 0)
                return o_d[ds(r0, P), ds(c0, 512)]

            for tp in range(1, TB // 2):
                xa = xcur.pop(tp)
                xb = xcur.pop(tp + 8)
                if tp + 1 < TB // 2:
                    for t in (tp + 1, tp + 9):
                        xcur[t] = x_pool.tile([P, KB, P], bf16, tag="x", name=f"x_{t}")
                        nc.sync.dma_start(xcur[t][:], xt_d[t])

                am = am_by_tp.pop(tp)
                osb = [
                    [osb_pool.tile([P, 512], f32, tag="osb", name=f"c_{tp}_{qd}_{q}")
                     for q in range(2)]
                    for qd in range(4)
                ]
                seen = {0: 0, 1: 0, 2: 0, 3: 0}
                for m in m_order:
                    psum = [
                        ps_pool.tile([P, 512], f32, tag="ps", name=f"ps_{tp}_{m}_{q}")
                        for q in range(2)
                    ]
                    for j in range(8):
                        for q in range(2):
                            nc.tensor.matmul(
                                psum[q][:],
                                lhsT=am[m][:, j, :],
                                rhs=rhs(m, j, q)[:],
                                start=(j == 0),
                                stop=(j == 7),
                            )
                    for qd, sgn in recipe[m]:
                        first = seen[qd] == 0
                        seen[qd] += 1
                        last = seen[qd] == n_ops[qd]
                        for q in range(2):
                            o = osb[qd][q]
                            if first:
                                if sgn > 0:
                                    nc.scalar.copy(o[:], psum[q][:])
                                else:
                                    nc.vector.tensor_scalar_mul(o[:], psum[q][:], -1.0)
                            else:
                                nc.vector.tensor_tensor(
                                    out=o[:], in0=o[:], in1=psum[q][:],
                                    op=add if sgn > 0 else sub,
                                )
                            if last:
                                eng = nc.scalar if (qd + q) % 2 == 0 else nc.gpsimd
                                eng.dma_start(out_ap(qd, tp, q), o[:])
                # build next pass's A-combos at end of pass (x has arrived;
                # keeps the DVE queue from blocking on the DMA)
                if tp + 1 < TB // 2:
                    am_by_tp[tp + 1] = build_am(tp + 1, xcur[tp + 1], xcur[tp + 9])

    nc.compile()
    return nc


def _get_compiled():
    global _cached
    if _cached is None:
        _cached = _build()
    return _cached


def _ensure_ntff_hook():
    import sys as _sys
    import types as _types

    if "antenv.axon_hooks" not in _sys.modules:
        import antenv

        mod = _types.ModuleType("antenv.axon_hooks")
        mod._hook = None

        def set_axon_ntff_profile_hook(h):
            mod._hook = h

        def get_axon_ntff_profile_hook():
            return mod._hook

        mod.set_axon_ntff_profile_hook = set_axon_ntff_profile_hook
        mod.get_axon_ntff_profile_hook = get_axon_ntff_profile_hook
        _sys.modules["antenv.axon_hooks"] = mod
        antenv.axon_hooks = mod
    mod = _sys.modules["antenv.axon_hooks"]
    if mod._hook is None:
        from trn_agent_boot.trn_boot import _ntff_profile_via_ctypes

        hook = _ntff_profile_via_ctypes("/opt/axon/libaxon_pjrt.so")
        if hook is not None:
            mod.set_axon_ntff_profile_hook(hook)


def run(x, weight, weight_scale, trace=False, trace_cores=None):
    from concourse.bass_utils import run_bass_kernel_spmd

    import ml_dtypes

    nc = _get_compiled()

    x = np.asarray(x, dtype=np.float32)
    weight = np.asarray(weight, dtype=np.float32)
    weight_scale = np.asarray(weight_scale, dtype=np.float32)

    wt = np.ascontiguousarray(
        weight.reshape(NCH, 512, KB, P).transpose(0, 2, 3, 1).astype(ml_dtypes.bfloat16)
    )
    scales_b = np.ascontiguousarray(
        np.broadcast_to(weight_scale.T[None, :, :], (P, KB, OBL)).astype(
            ml_dtypes.bfloat16
        )
    )

    in_maps = []
    for c in range(NCORES):
        xs = x[c * TSH : (c + 1) * TSH]
        xt = np.ascontiguousarray(
            xs.reshape(TB, P, KB, P).transpose(0, 3, 2, 1).astype(ml_dtypes.bfloat16)
        )
        in_maps.append({"xt": xt, "wt": wt, "s": scales_b})

    kwargs = {}
    if trace:
        try:
            _ensure_ntff_hook()
        except Exception as e:
            print(f"ntff hook registration failed ({e}); tracing may be skipped")
        kwargs = dict(trace=True, trace_cores=trace_cores or [0])
    res = run_bass_kernel_spmd(nc, in_maps, core_ids=list(range(NCORES)), **kwargs)
    out = np.concatenate([res.results[c]["out"] for c in range(NCORES)], axis=0)
    return out, res


def kernel(x, weight, weight_scale):
    try:
        out, _ = run(x, weight, weight_scale)
    except Exception:
        import time

        time.sleep(2)
        out, _ = run(x, weight, weight_scale)
    return out


# revision 12
# speedup vs baseline: 1.0787x; 1.0787x over previous
"""Hybrid Strassen TRN2 kernel for MockFP8Linear: out = x @ (W*scale)^T.

Per core: C [2048,2048] = A @ B, A = x-shard, B = (W*scale)^T.
Token rows tt0/tt8 are computed as a PLAIN chunk-pair matmul stream first
(no combos needed -- it runs while the B-side Strassen operands are being
dequantized and combined), then one Strassen level handles the remaining
7 row-pair passes: 4*32 + 7*112 = 912 N=512 matmuls vs 1024 plain (-10.9%).

  M1=(A11+A22)(B11+B22)  M2=(A21+A22)B11      M3=A11(B12-B22)
  M4=A22(B21-B11)        M5=(A11+A12)B22      M6=(A21-A11)(B11+B12)
  M7=(A12-A22)(B21+B22)
  C11=M1+M4-M5+M7  C12=M3+M5  C21=M2+M4  C22=M1-M2+M3+M6

All matmuls are PAIRED: one stationary lhsT feeds 2 matmuls into 2
alternating PSUM banks (per-matmul stationary swaps measured +43 ns each).
Process order per Strassen pass: [M2, M5, M1, M7, M4, M3, M6]; each
C-quadrant accumulator's first contribution is a copy (ACT) or negated
copy (DVE tensor_scalar_mul), later ones accumulate in place on DVE.

B quadrants (r = i-half, q = o-half): B_rq = wsb[ch in {2q,2q+1}][ib in
8r..8r+8). cB3=B12-B22 / cB4=B21-B11 are computed IN PLACE over the raw
B12/B21 slabs (raw values are only read by the plain pass and cb6/cb7,
which Tile orders first). cb1/cb6/cb7 get their own tiles (6 MB).
Engine budget: DVE = dequants, cb4/cb6/cb3, A-combos, accumulate ops;
GPSIMD = cb1/cb7 (+output DMA issues); ACT = plain copies (+output DMAs).
x slabs stream through a 4-buffer pool (each pass uses only tt', tt'+8).
"""

import os
import sys

import numpy as np

for _p in ("/opt/trn_rl_repo", "/root/.axon_site/_ro/trn_rl_repo"):
    if os.path.isdir(_p) and _p not in sys.path:
        sys.path.append(_p)

TOKENS, IN_F, OUT_F = 16384, 2048, 2048
NCORES = 8
TSH = TOKENS // NCORES
P = 128
KB = IN_F // P
TB = TSH // P
OBL = OUT_F // P
NCH = OUT_F // 512

_cached = None


def _build():
    from contextlib import ExitStack

    import concourse.tile as tile
    from concourse import bacc, mybir
    from concourse.bass import ds

    f32 = mybir.dt.float32
    bf16 = mybir.dt.bfloat16
    add = mybir.AluOpType.add
    sub = mybir.AluOpType.subtract
    mult = mybir.AluOpType.mult

    nc = bacc.Bacc("TRN2", target_bir_lowering=False, debug=False, num_devices=NCORES)
    xt_d = nc.dram_tensor("xt", [TB, P, KB, P], bf16, kind="ExternalInput").ap()
    wt_d = nc.dram_tensor("wt", [NCH, KB, P, 512], bf16, kind="ExternalInput").ap()
    s_d = nc.dram_tensor("s", [P, KB, OBL], bf16, kind="ExternalInput").ap()
    o_d = nc.dram_tensor("out", [TSH, OUT_F], f32, kind="ExternalOutput").ap()

    with tile.TileContext(nc) as tc:
        with ExitStack() as ctx:
            const = ctx.enter_context(tc.tile_pool(name="const", bufs=1))
            scales = const.tile([P, KB, OBL], bf16)
            nc.sync.dma_start(scales[:], s_d[:])

            w_pool = ctx.enter_context(tc.tile_pool(name="w", bufs=1))
            wsb = [
                [w_pool.tile([P, 512], bf16, name=f"w_{ch}_{ib}") for ib in range(KB)]
                for ch in range(NCH)
            ]
            cb_pool = ctx.enter_context(tc.tile_pool(name="cb", bufs=1))
            cb1 = [
                [cb_pool.tile([P, 512], bf16, name=f"cb1_{j}_{q}") for q in range(2)]
                for j in range(8)
            ]
            cb6 = [
                [cb_pool.tile([P, 512], bf16, name=f"cb6_{j}_{q}") for q in range(2)]
                for j in range(8)
            ]
            cb7 = [
                [cb_pool.tile([P, 512], bf16, name=f"cb7_{j}_{q}") for q in range(2)]
                for j in range(8)
            ]

            x_pool = ctx.enter_context(tc.tile_pool(name="x", bufs=8))
            am_pool = ctx.enter_context(tc.tile_pool(name="am", bufs=10))
            osb_pool = ctx.enter_context(tc.tile_pool(name="osb", bufs=8))
            osbp_pool = ctx.enter_context(tc.tile_pool(name="osbp", bufs=4))
            ps_pool = ctx.enter_context(tc.tile_pool(name="ps", bufs=8, space="PSUM"))

            # PE warmup (flips the HAM clock gate during the preamble)
            wm_ps = ps_pool.tile([16, 256], f32, tag="ps", name="warm")
            for _ in range(7):
                nc.tensor.matmul(
                    wm_ps[:], lhsT=scales[:, 0, :], rhs=scales[:, :, :],
                    start=True, stop=True,
                )

            def load_w(ch, ib):
                nc.sync.dma_start(wsb[ch][ib][:], wt_d[ch, ib])
                nc.vector.tensor_tensor(
                    out=wsb[ch][ib][:].rearrange("p (b c) -> p b c", c=P),
                    in0=wsb[ch][ib][:].rearrange("p (b c) -> p b c", c=P),
                    in1=scales[:, ib, ds(ch * 4, 4), None].broadcast_to([P, 4, P]),
                    op=mult,
                )

            B11 = lambda j, q: wsb[q][j]
            B21 = lambda j, q: wsb[q][8 + j]
            B22 = lambda j, q: wsb[2 + q][8 + j]
            B12 = lambda j, q: wsb[2 + q][j]

            # ---- W DMA+dequant in chunk-pair JIT order (plain pass shape),
            # x slabs 0/8 woven in early ----
            N_PLAIN = 3  # token-pairs computed plainly while B-prep runs
            load_w(0, 0)
            load_w(1, 0)
            xcur = {}
            for t in (0, 1, 2, 8, 9, 10):
                xcur[t] = x_pool.tile([P, KB, P], bf16, tag="x", name=f"x_{t}")
            nc.sync.dma_start(xcur[0][:], xt_d[0])
            for ib in range(1, KB):
                load_w(0, ib)
                load_w(1, ib)
                if ib in (4, 8, 12):
                    nc.sync.dma_start(xcur[{4: 8, 8: 1, 12: 9}[ib]][:],
                                      xt_d[{4: 8, 8: 1, 12: 9}[ib]])
            for ch in (2, 3):
                for ib in range(KB):
                    load_w(ch, ib)
            nc.sync.dma_start(xcur[2][:], xt_d[2])
            nc.sync.dma_start(xcur[10][:], xt_d[10])

            # ---- plain chunk-pair pass over token tiles 0 and 8 ----
            def plain_group(chp, tt, xa):
                psum = [
                    ps_pool.tile([P, 512], f32, tag="ps", name=f"pp_{chp}_{tt}_{k}")
                    for k in range(2)
                ]
                for ib in range(KB):
                    for k in range(2):
                        nc.tensor.matmul(
                            psum[k][:],
                            lhsT=xa[:, ib, :],
                            rhs=wsb[2 * chp + k][ib][:],
                            start=(ib == 0),
                            stop=(ib == KB - 1),
                        )
                osb = osbp_pool.tile([P, 1024], f32, tag="osbp", name=f"op_{chp}_{tt}")
                nc.scalar.copy(osb[:, ds(0, 512)], psum[0][:])
                nc.scalar.copy(osb[:, ds(512, 512)], psum[1][:])
                nc.gpsimd.dma_start(o_d[ds(tt * P, P), ds(chp * 1024, 1024)], osb[:])

            for pp in range(N_PLAIN):
                for chp in range(2):
                    for tt in (pp, pp + 8):
                        plain_group(chp, tt, xcur[tt])

            # prefetch x for the first strassen pass
            for t in (N_PLAIN, N_PLAIN + 8):
                xcur[t] = x_pool.tile([P, KB, P], bf16, tag="x", name=f"x_{t}")
                nc.sync.dma_start(xcur[t][:], xt_d[t])

            # ---- B-combos (built while the plain pass runs) ----
            # GPSIMD: cb1, cb7 (pure reads). DVE: cb4, cb6, cb3 (cb4/cb3 in
            # place; Tile orders them after the plain pass + cb6/cb7 reads).
            for j in range(8):
                for q in range(2):
                    nc.gpsimd.tensor_tensor(
                        out=cb1[j][q][:], in0=B11(j, q)[:], in1=B22(j, q)[:], op=add
                    )
            for j in range(8):
                for q in range(2):
                    nc.gpsimd.tensor_tensor(
                        out=cb7[j][q][:], in0=B21(j, q)[:], in1=B22(j, q)[:], op=add
                    )
            for j in range(8):
                for q in range(2):
                    nc.vector.tensor_tensor(
                        out=B21(j, q)[:], in0=B21(j, q)[:], in1=B11(j, q)[:], op=sub
                    )  # cb4 (after cb7 read raw B21)
            for j in range(8):
                for q in range(2):
                    nc.vector.tensor_tensor(
                        out=cb6[j][q][:], in0=B11(j, q)[:], in1=B12(j, q)[:], op=add
                    )
            for j in range(8):
                for q in range(2):
                    nc.vector.tensor_tensor(
                        out=B12(j, q)[:], in0=B12(j, q)[:], in1=B22(j, q)[:], op=sub
                    )  # cb3 (after cb6 read raw B12)

            def rhs(m, j, q):
                return {
                    1: cb1[j][q], 2: B11(j, q), 3: B12(j, q), 4: B21(j, q),
                    5: B22(j, q), 6: cb6[j][q], 7: cb7[j][q],
                }[m]

            def build_am(tp, xa, xb):
                a11 = xa[:, ds(0, 8), :]
                a12 = xa[:, ds(8, 8), :]
                a21 = xb[:, ds(0, 8), :]
                a22 = xb[:, ds(8, 8), :]
                am = {3: a11, 4: a22}
                for m, (i0, i1, op) in {
                    1: (a11, a22, add),
                    2: (a21, a22, add),
                    5: (a11, a12, add),
                    6: (a21, a11, sub),
                    7: (a12, a22, sub),
                }.items():
                    t = am_pool.tile([P, 8, P], bf16, tag="am", name=f"am_{tp}_{m}")
                    nc.vector.tensor_tensor(out=t[:], in0=i0, in1=i1, op=op)
                    am[m] = t
                return am

            am_by_tp = {N_PLAIN: build_am(N_PLAIN, xcur[N_PLAIN], xcur[N_PLAIN + 8])}

            # C assembly: osb 0=C11, 1=C12, 2=C21, 3=C22
            recipe = {
                1: [(0, +1), (3, +1)],
                2: [(2, +1), (3, -1)],
                3: [(1, +1), (3, +1)],
                4: [(0, +1), (2, +1)],
                5: [(0, -1), (1, +1)],
                6: [(3, +1)],
                7: [(0, +1)],
            }
            m_order = [2, 5, 1, 7, 4, 3, 6]
            n_ops = {0: 4, 1: 2, 2: 2, 3: 4}

            def out_ap(qd, tp, q):
                r0 = tp * P + (1024 if qd >= 2 else 0)
                c0 = q * 512 + (1024 if qd % 2 == 1 else 0)
                return o_d[ds(r0, P), ds(c0, 512)]

            for tp in range(N_PLAIN, TB // 2):
                xa = xcur.pop(tp)
                xb = xcur.pop(tp + 8)
                if tp + 1 < TB // 2:
                    for t in (tp + 1, tp + 9):
                        xcur[t] = x_pool.tile([P, KB, P], bf16, tag="x", name=f"x_{t}")
                        nc.sync.dma_start(xcur[t][:], xt_d[t])

                am = am_by_tp.pop(tp)
                osb = [
                    [osb_pool.tile([P, 512], f32, tag="osb", name=f"c_{tp}_{qd}_{q}")
                     for q in range(2)]
                    for qd in range(4)
                ]
                seen = {0: 0, 1: 0, 2: 0, 3: 0}
                for m in m_order:
                    psum = [
                        ps_pool.tile([P, 512], f32, tag="ps", name=f"ps_{tp}_{m}_{q}")
                        for q in range(2)
                    ]
                    for j in range(8):
                        for q in range(2):
                            nc.tensor.matmul(
                                psum[q][:],
                                lhsT=am[m][:, j, :],
                                rhs=rhs(m, j, q)[:],
                                start=(j == 0),
                                stop=(j == 7),
                            )
                    for qd, sgn in recipe[m]:
                        first = seen[qd] == 0
                        seen[qd] += 1
                        last = seen[qd] == n_ops[qd]
                        for q in range(2):
                            o = osb[qd][q]
                            if first:
                                if sgn > 0:
                                    nc.scalar.copy(o[:], psum[q][:])
                                else:
                                    nc.vector.tensor_scalar_mul(o[:], psum[q][:], -1.0)
                            else:
                                nc.vector.tensor_tensor(
                                    out=o[:], in0=o[:], in1=psum[q][:],
                                    op=add if sgn > 0 else sub,
                                )
                            if last:
                                eng = nc.scalar if (qd + q) % 2 == 0 else nc.gpsimd
                                eng.dma_start(out_ap(qd, tp, q), o[:])
                # build next pass's A-combos at end of pass (x has arrived;
                # keeps the DVE queue from blocking on the DMA)
                if tp + 1 < TB // 2:
                    am_by_tp[tp + 1] = build_am(tp + 1, xcur[tp + 1], xcur[tp + 9])

    nc.compile()
    return nc


def _get_compiled():
    global _cached
    if _cached is None:
        _cached = _build()
    return _cached


def _ensure_ntff_hook():
    import sys as _sys
    import types as _types

    if "antenv.axon_hooks" not in _sys.modules:
        import antenv

        mod = _types.ModuleType("antenv.axon_hooks")
        mod._hook = None

        def set_axon_ntff_profile_hook(h):
            mod._hook = h

        def get_axon_ntff_profile_hook():
            return mod._hook

        mod.set_axon_ntff_profile_hook = set_axon_ntff_profile_hook
        mod.get_axon_ntff_profile_hook = get_axon_ntff_profile_hook
        _sys.modules["antenv.axon_hooks"] = mod
        antenv.axon_hooks = mod
    mod = _sys.modules["antenv.axon_hooks"]
    if mod._hook is None:
        from trn_agent_boot.trn_boot import _ntff_profile_via_ctypes

        hook = _ntff_profile_via_ctypes("/opt/axon/libaxon_pjrt.so")
        if hook is not None:
            mod.set_axon_ntff_profile_hook(hook)


def run(x, weight, weight_scale, trace=False, trace_cores=None):
    from concourse.bass_utils import run_bass_kernel_spmd

    import ml_dtypes

    nc = _get_compiled()

    x = np.asarray(x, dtype=np.float32)
    weight = np.asarray(weight, dtype=np.float32)
    weight_scale = np.asarray(weight_scale, dtype=np.float32)

    wt = np.ascontiguousarray(
        weight.reshape(NCH, 512, KB, P).transpose(0, 2, 3, 1).astype(ml_dtypes.bfloat16)
    )
    scales_b = np.ascontiguousarray(
        np.broadcast_to(weight_scale.T[None, :, :], (P, KB, OBL)).astype(
            ml_dtypes.bfloat16
        )
    )

    in_maps = []
    for c in range(NCORES):
        xs = x[c * TSH : (c + 1) * TSH]
        xt = np.ascontiguousarray(
            xs.reshape(TB, P, KB, P).transpose(0, 3, 2, 1).astype(ml_dtypes.bfloat16)
        )
        in_maps.append({"xt": xt, "wt": wt, "s": scales_b})

    kwargs = {}
    if trace:
        try:
            _ensure_ntff_hook()
        except Exception as e:
            print(f"ntff hook registration failed ({e}); tracing may be skipped")
        kwargs = dict(trace=True, trace_cores=trace_cores or [0])
    res = run_bass_kernel_spmd(nc, in_maps, core_ids=list(range(NCORES)), **kwargs)
    out = np.concatenate([res.results[c]["out"] for c in range(NCORES)], axis=0)
    return out, res


def kernel(x, weight, weight_scale):
    try:
        out, _ = run(x, weight, weight_scale)
    except Exception:
        import time

        time.sleep(2)
        out, _ = run(x, weight, weight_scale)
    return out


# revision 14
# speedup vs baseline: 1.1432x; 1.0598x over previous
"""Trainium2 Bass kernel for MockFP8Linear: out = x @ (W * block_scale)^T.

Strategy: data-parallel over tokens across 8 NeuronCores (no collectives).

v2: pure bf16 matmul stream. Both operands are fed to the device already in
the [contraction-on-partitions] layout (host-side np transpose + bf16 cast,
same prep class as the W^T layout prep the baseline already used), so the
PE does nothing but the 1024 N=512 matmuls per core:

  - x: host-prepped to xt[tt, i, ib, t] = x[tt*128+t, ib*128+i] (bf16), so
    each token tile tt is one contiguous 512 KB slab whose SBUF layout is
    [128(i) partitions, 16(ib) x 128(t)]. lhsT for (tt, ib) is the [128,128]
    slice [:, ib, :].
  - weight: host-prepped to wt[ch, ib, i, o] = W[ch*512+o, ib*128+i] (bf16):
    64 contiguous 128 KB slabs. The per-128x128-block dequant scaling stays
    on-device: one DVE tensor_tensor multiply per slab (in-place, scale
    broadcast along o within each 128-block).

Main loop is ch-major (4 output column chunks of 512 x 16 token tiles):
the first chunk's working set is only 2 MB of W + streaming x slabs, so
real matmuls start as soon as the first slabs land (~4 us) instead of
waiting for the full 8 MB W. Each (ch, tt) group accumulates 16 matmuls
into one PSUM bank (8-bank rotation), then DVE/ACT alternate evictions to
SBUF and issue the 256 KB output DMA. Warm steady state measured at the
215.8 ns N=512 issue floor -> ~221 us of PE busy per core.

DMA issue order (all inputs from the sync queue, outputs from the evicting
engine's queue): w[ch0][0], xt[0], w[ch0][1..15], xt[1..15], w[ch1..ch3].
"""

import os
import sys

import numpy as np

for _p in ("/opt/trn_rl_repo", "/root/.axon_site/_ro/trn_rl_repo"):
    if os.path.isdir(_p) and _p not in sys.path:
        sys.path.append(_p)

TOKENS, IN_F, OUT_F = 16384, 2048, 2048
NCORES = 8
TSH = TOKENS // NCORES  # tokens per core
P = 128
KB = IN_F // P  # contraction blocks (ib)
TB = TSH // P  # token tiles per core (tt)
OBL = OUT_F // P  # out_features blocks (scale granularity)
NCH = OUT_F // 512  # output column chunks (ch)

_cached = None


def _build():
    from contextlib import ExitStack

    import concourse.tile as tile
    from concourse import bacc, mybir
    from concourse.bass import ds

    f32 = mybir.dt.float32
    bf16 = mybir.dt.bfloat16

    nc = bacc.Bacc("TRN2", target_bir_lowering=False, debug=False, num_devices=NCORES)
    xt_d = nc.dram_tensor("xt", [TB, P, KB, P], bf16, kind="ExternalInput").ap()
    wt_d = nc.dram_tensor("wt", [NCH, KB, P, 512], bf16, kind="ExternalInput").ap()
    s_d = nc.dram_tensor("s", [P, KB, OBL], bf16, kind="ExternalInput").ap()
    o_d = nc.dram_tensor("out", [TSH, OUT_F], f32, kind="ExternalOutput").ap()

    with tile.TileContext(nc) as tc:
        with ExitStack() as ctx:
            const = ctx.enter_context(tc.tile_pool(name="const", bufs=1))
            scales = const.tile([P, KB, OBL], bf16)
            nc.sync.dma_start(scales[:], s_d[:])

            w_pool = ctx.enter_context(tc.tile_pool(name="w", bufs=1))
            wsb = [
                [w_pool.tile([P, 512], bf16, name=f"w_{ch}_{ib}") for ib in range(KB)]
                for ch in range(NCH)
            ]
            x_pool = ctx.enter_context(tc.tile_pool(name="x", bufs=1))
            xsb = [x_pool.tile([P, KB, P], bf16, name=f"x_{tt}") for tt in range(TB)]

            osb_pool = ctx.enter_context(tc.tile_pool(name="osb", bufs=4))
            ps_pool = ctx.enter_context(tc.tile_pool(name="ps", bufs=8, space="PSUM"))

            # ---- PE warmup: a few dummy f32 matmuls on the scales tile
            # (values irrelevant, result never read) make the PE busy during
            # the runtime preamble/input-DMA window so the HAM clock gate
            # unthrottles to 2.4 GHz before the real stream starts. ----
            wm_ps = ps_pool.tile([16, 256], f32, tag="ps", name="warm")
            for _ in range(18):
                nc.tensor.matmul(
                    wm_ps[:],
                    lhsT=scales[:, 0, :],
                    rhs=scales[:, :, :],
                    start=True,
                    stop=True,
                )

            def load_w(ch, ib):
                nc.sync.dma_start(wsb[ch][ib][:], wt_d[ch, ib])
                # in-place dequant: scale block bo = ch*4 + (o//128).
                # DVE with bf16 scales; evictions live on ACT so the DVE
                # dequant backlog never stalls PSUM recycling.
                eng = nc.vector
                eng.tensor_tensor(
                    out=wsb[ch][ib][:].rearrange("p (b c) -> p b c", c=P),
                    in0=wsb[ch][ib][:].rearrange("p (b c) -> p b c", c=P),
                    in1=scales[:, ib, ds(ch * 4, 4), None].broadcast_to([P, 4, P]),
                    op=mybir.AluOpType.mult,
                )

            # ---- input DMA issue order: first chunk-pair's W + x slab 0
            # first, then the remaining x slabs (JIT for the chp0 pass),
            # then the rest of W (needed only at ~110 us into the stream).
            load_w(0, 0)
            load_w(1, 0)
            nc.sync.dma_start(xsb[0][:], xt_d[0])
            for ib in range(1, KB):
                load_w(0, ib)
                load_w(1, ib)
                # weave the first few x slabs between the W pairs so the
                # early token-tile groups never wait (a single early PE gap
                # resets the HAM busy window and keeps the clock at 1.2 GHz)
                if ib % 5 == 0 and ib // 5 < 4:
                    nc.sync.dma_start(xsb[ib // 5][:], xt_d[ib // 5])
            for tt in range(4, TB):
                nc.sync.dma_start(xsb[tt][:], xt_d[tt])
            for ch in range(2, NCH):
                for ib in range(KB):
                    load_w(ch, ib)

            # chunk-pair passes: each lhsT (stationary) feeds 2 matmuls
            # into 2 alternating PSUM banks, like the fastest measured
            # stream shape (stationary reuse + bank alternation).
            for chp in range(NCH // 2):
                for tt in range(TB):
                    psum = [
                        ps_pool.tile([P, 512], f32, tag="ps", name=f"ps_{chp}_{tt}_{k}")
                        for k in range(2)
                    ]
                    for ib in range(KB):
                        for k in range(2):
                            nc.tensor.matmul(
                                psum[k][:],
                                lhsT=xsb[tt][:, ib, :],
                                rhs=wsb[2 * chp + k][ib][:],
                                start=(ib == 0),
                                stop=(ib == KB - 1),
                            )
                    osb = osb_pool.tile([P, 1024], f32, tag="osb", name=f"o_{chp}_{tt}")
                    last2 = chp == NCH // 2 - 1 and tt >= TB - 2
                    if last2:
                        # drain fast: split evictions DVE/ACT (the DVE
                        # dequant queue is long empty) and 4 quarter-DMAs
                        # across queue engines
                        nc.vector.tensor_copy(osb[:, ds(0, 512)], psum[0][:])
                        nc.scalar.copy(osb[:, ds(512, 512)], psum[1][:])
                        for qtr, qeng in enumerate(
                            (nc.sync, nc.gpsimd, nc.scalar, nc.sync)
                        ):
                            qeng.dma_start(
                                o_d[ds(tt * P, P), ds(chp * 1024 + qtr * 256, 256)],
                                osb[:, ds(qtr * 256, 256)],
                            )
                    else:
                        # both evictions on ACT (DVE holds the dequant queue)
                        nc.scalar.copy(osb[:, ds(0, 512)], psum[0][:])
                        nc.scalar.copy(osb[:, ds(512, 512)], psum[1][:])
                        nc.gpsimd.dma_start(
                            o_d[ds(tt * P, P), ds(chp * 1024, 1024)], osb[:]
                        )

    nc.compile()
    return nc


def _get_compiled():
    global _cached
    if _cached is None:
        _cached = _build()
    return _cached


def _ensure_ntff_hook():
    """Register the axon NTFF profile hook (boot skips it when
    antenv.axon_hooks is absent from the image). Only needed for trace=True."""
    import sys as _sys
    import types as _types

    if "antenv.axon_hooks" not in _sys.modules:
        import antenv

        mod = _types.ModuleType("antenv.axon_hooks")
        mod._hook = None

        def set_axon_ntff_profile_hook(h):
            mod._hook = h

        def get_axon_ntff_profile_hook():
            return mod._hook

        mod.set_axon_ntff_profile_hook = set_axon_ntff_profile_hook
        mod.get_axon_ntff_profile_hook = get_axon_ntff_profile_hook
        _sys.modules["antenv.axon_hooks"] = mod
        antenv.axon_hooks = mod
    mod = _sys.modules["antenv.axon_hooks"]
    if mod._hook is None:
        from trn_agent_boot.trn_boot import _ntff_profile_via_ctypes

        hook = _ntff_profile_via_ctypes("/opt/axon/libaxon_pjrt.so")
        if hook is not None:
            mod.set_axon_ntff_profile_hook(hook)


def run(x, weight, weight_scale, trace=False, trace_cores=None):
    from concourse.bass_utils import run_bass_kernel_spmd

    import ml_dtypes

    nc = _get_compiled()

    x = np.asarray(x, dtype=np.float32)
    weight = np.asarray(weight, dtype=np.float32)
    weight_scale = np.asarray(weight_scale, dtype=np.float32)

    # wt[ch, ib, i, o] = W[ch*512+o, ib*128+i]
    wt = np.ascontiguousarray(
        weight.reshape(NCH, 512, KB, P).transpose(0, 2, 3, 1).astype(ml_dtypes.bfloat16)
    )
    # s[p, bi, bo] = weight_scale[bo, bi] broadcast over partitions
    scales_b = np.ascontiguousarray(
        np.broadcast_to(weight_scale.T[None, :, :], (P, KB, OBL)).astype(
            ml_dtypes.bfloat16
        )
    )

    in_maps = []
    for c in range(NCORES):
        xs = x[c * TSH : (c + 1) * TSH]
        # xt[tt, i, ib, t] = xs[tt*128+t, ib*128+i]
        xt = np.ascontiguousarray(
            xs.reshape(TB, P, KB, P).transpose(0, 3, 2, 1).astype(ml_dtypes.bfloat16)
        )
        in_maps.append({"xt": xt, "wt": wt, "s": scales_b})

    kwargs = {}
    if trace:
        try:
            _ensure_ntff_hook()
        except Exception as e:  # tracing is best-effort; the run still works
            print(f"ntff hook registration failed ({e}); tracing may be skipped")
        kwargs = dict(trace=True, trace_cores=trace_cores or [0])
    res = run_bass_kernel_spmd(nc, in_maps, core_ids=list(range(NCORES)), **kwargs)
    out = np.concatenate([res.results[c]["out"] for c in range(NCORES)], axis=0)
    return out, res


def kernel(x, weight, weight_scale):
    # Rare transient device errors (NRT_EXEC_UNIT_UNRECOVERABLE) have been
    # observed under the profiling path; retry once to be safe.
    try:
        out, _ = run(x, weight, weight_scale)
    except Exception:
        import time

        time.sleep(2)
        out, _ = run(x, weight, weight_scale)
    return out


if __name__ == "__main__":
    pass
